# revision 14
# baseline (speedup 1.0000x reference)
"""ConvDecoder Bass kernel for Trainium2, SPMD over 8 NeuronCores.

Math (per batch element b, one per core):
    r_conv = Conv1d(r, conv_w, SAME) + conv_b            # (C, N_IN)
    d[n,m] = (xc[n] - xt[m])^2                           # (N_IN, N_OUT)
    wt_c   = exp(-0.5 * d / exp(sigma_c)^2)
    z[m,c] = sum_n r_conv[c,n] * wt_c[n,m]
    out    = z @ lin_w.T + lin_b                         # (N_OUT, OUT_C)

Per-core structure (v2):
  - Conv1d as an im2col matmul: ones row (bias) + 5 shifted DMA copies of r
    stacked on partitions -> (81, 512); matmul with repacked weights
    (81, 16) yields the conv output directly in (n, c) layout = the lhsT
    of the RBF-reduction matmul.
  - xt is partition-broadcast by a stride-0 DMA; GpSimd computes
    diff = xt - xc_p (per-partition scalar), DVE/ACT square it, ACT
    exponentiates with scale=-a (a = 0.5/scale^2, baked per group) ->
    E chunk (128, 512). fp32 end to end.
  - z[c,m] over the 4 n-tiles: 4 matmuls issued to 4 distinct PE column
    strips (tile_position) run concurrently in the array; DVE reduces the
    4 PSUM partials. (Channels sharing a length scale share one E map;
    with >1 sigma group, falls back to sequential PSUM accumulation.)
  - Final linear: (16,128)^T @ (16,32) matmul per m-tile; lin_b is folded
    into the PSUM->SBUF copy as a DVE add against a broadcast row.
"""

import numpy as np

import concourse.bass as bass
import concourse.mybir as mybir
from concourse.tile import TileContext
from concourse.bass_utils import run_bass_kernel_spmd

F32 = mybir.dt.float32

B, N_IN, N_OUT, C, OUT_C, KW = 8, 512, 1024, 16, 32, 5
N_CORES = 8
NT = N_IN // 128   # n tiles (4)
MH = N_OUT // 512  # m halves (2)
MT = 512 // 128    # m tiles per half (4)

# chunks whose square runs on ACT (Square) instead of DVE, to balance
# engine load: DVE also does the strip-reduce, ACT also does the exps.
SQ_ON_ACT = {3}


# --- walrus workaround -----------------------------------------------------
# This container's walrus accepts at most ONE semaphore wait per TPB
# instruction, but Tile's scheduler attaches several (joins + tail drain).
# Hoist all but the last wait of each instruction onto fresh wait-only
# EventSemaphore instructions inserted right before it on the same engine.
_ws_ctr = [0]


def _split_multi_waits(nc):
    for fn in nc.m.functions:
        for blk in fn.blocks:
            insts = blk.instructions
            if not any(
                ins.sync_info and len(ins.sync_info.on_wait) > 1 for ins in insts
            ):
                continue
            out = []
            for ins in insts:
                si = ins.sync_info
                waits = list(si.on_wait) if si else []
                if len(waits) > 1:
                    for w in waits[:-1]:
                        _ws_ctr[0] += 1
                        ev = mybir.InstEventSemaphore(
                            name=f"waitsplit_{_ws_ctr[0]}", ins=[], outs=[]
                        )
                        ev.engine = ins.engine
                        ev.sync_info = mybir.SyncInfo(on_wait=[w], on_update=[])
                        nc.register_instruction(ev)
                        out.append(ev)
                    ins.sync_info = mybir.SyncInfo(
                        on_wait=[waits[-1]], on_update=list(si.on_update)
                    )
                out.append(ins)
            insts[:] = out


# --- kernel build ----------------------------------------------------------
def _build(groups):
    """groups: tuple of (c0, c1, a) with contiguous channel ranges."""
    nc = bass.Bass()
    r_in = nc.dram_tensor("r", [C, N_IN], F32, kind="ExternalInput")
    xc_in = nc.dram_tensor("xc", [1, N_IN], F32, kind="ExternalInput")
    xt_in = nc.dram_tensor("xt", [1, N_OUT], F32, kind="ExternalInput")
    wconv = nc.dram_tensor("w_aug", [C * KW + 1, C], F32, kind="ExternalInput")
    wlin = nc.dram_tensor("lin_w_t", [C, OUT_C], F32, kind="ExternalInput")
    blin = nc.dram_tensor("lin_b", [1, OUT_C], F32, kind="ExternalInput")
    y_out = nc.dram_tensor("y", [N_OUT, OUT_C], F32, kind="ExternalOutput")

    Exp = mybir.ActivationFunctionType.Exp
    Square = mybir.ActivationFunctionType.Square
    single_group = len(groups) == 1

    with TileContext(nc) as tc:
        with (
            tc.tile_pool(name="const", bufs=1) as cpool,
            tc.tile_pool(name="work", bufs=1) as wpool,
            tc.tile_pool(name="psum", bufs=1, space="PSUM") as ppool,
        ):
            # ---- constants / operand rows ----
            wa = cpool.tile([C * KW + 1, C], F32)
            nc.sync.dma_start(out=wa[:], in_=wconv[:])
            wl = cpool.tile([C, OUT_C], F32)
            nc.sync.dma_start(out=wl[:], in_=wlin[:])
            blb = cpool.tile([128, OUT_C], F32)
            nc.sync.dma_start(out=blb[:], in_=blin[0:1, :].partition_broadcast(128))

            # xc as per-partition scalars: xc_pt[p, t] = xc[t*128 + p]
            xc_pt = cpool.tile([128, NT], F32)
            nc.sync.dma_start(
                out=xc_pt[:], in_=xc_in[0, :].rearrange("(t p) -> p t", p=128)
            )
            # xt broadcast to all partitions, one tile per m-half
            xtb = []
            for mh in range(MH):
                t = cpool.tile([128, 512], F32, name=f"xtb{mh}")
                nc.sync.dma_start(
                    out=t[:],
                    in_=xt_in[0:1, mh * 512 : (mh + 1) * 512].partition_broadcast(128),
                )
                xtb.append(t)

            # ---- conv im2col stack: row 0 = ones (bias), rows 1+16k+ci ----
            stack = cpool.tile([C * KW + 1, N_IN], F32)
            nc.vector.memset(stack[:, :], 0.0)
            pad = KW // 2
            for k in range(KW):
                lo = max(0, pad - k)
                hi = min(N_IN, N_IN + pad - k)
                nc.sync.dma_start(
                    out=stack[1 + C * k : 1 + C * (k + 1), lo:hi],
                    in_=r_in[:, lo + k - pad : hi + k - pad],
                )
            nc.vector.memset(stack[0:1, :], 1.0)

            # ---- conv matmuls: (81,128)^T @ (81,16) -> (128,16) per n-tile ----
            r_t = []
            for t in range(NT):
                cps = ppool.tile([128, C], F32, tag="smallps", bufs=2,
                                 name=f"cps{t}")
                nc.tensor.matmul(
                    cps[:],
                    lhsT=stack[:, t * 128 : (t + 1) * 128],
                    rhs=wa[:],
                    start=True,
                    stop=True,
                )
                rsb = cpool.tile([128, C], F32, name=f"rsb{t}")
                nc.vector.tensor_copy(out=rsb[:], in_=cps[:])
                r_t.append(rsb)

            # ---- main pipeline over m-halves / n-tiles ----
            for mh in range(MH):
                z_sb = wpool.tile([C, 512], F32, tag="zsb", bufs=2, name=f"z{mh}")
                if single_group:
                    a0 = groups[0][2]
                    z4 = ppool.tile([128, 512], F32, tag="z4", bufs=2,
                                    name=f"z4_{mh}")
                    for k in range(NT):
                        diff = wpool.tile([128, 512], F32, tag="diff", bufs=3,
                                          name=f"diff{mh}_{k}")
                        nc.gpsimd.tensor_scalar(
                            diff[:], xtb[mh][:], xc_pt[:, k : k + 1], None,
                            op0=mybir.AluOpType.subtract,
                        )
                        dsq = wpool.tile([128, 512], F32, tag="dsq", bufs=3,
                                         name=f"dsq{mh}_{k}")
                        if k in SQ_ON_ACT:
                            nc.scalar.activation(dsq[:], diff[:], Square)
                        else:
                            nc.vector.tensor_mul(out=dsq[:], in0=diff[:],
                                                 in1=diff[:])
                        esb = wpool.tile([128, 512], F32, tag="esb", bufs=3,
                                         name=f"e{mh}_{k}")
                        nc.scalar.activation(esb[:], dsq[:], Exp,
                                             scale=-float(a0))
                        # one PE column strip per n-tile: the 4 matmuls run
                        # concurrently in the array
                        nc.tensor.matmul(
                            z4[32 * k : 32 * k + C, :],
                            lhsT=r_t[k][:],
                            rhs=esb[:],
                            start=True,
                            stop=True,
                            tile_position=(0, 32 * k),
                        )
                    # reduce the 4 strips (TT may read only one PSUM input)
                    acc = wpool.tile([C, 512], F32, tag="acc", bufs=2,
                                     name=f"acc{mh}")
                    nc.vector.tensor_copy(out=acc[:], in_=z4[0:C, :])
                    acc2 = wpool.tile([C, 512], F32, tag="acc2", bufs=2,
                                      name=f"acc2_{mh}")
                    nc.vector.tensor_add(out=acc2[:], in0=acc[:],
                                         in1=z4[32 : 32 + C, :])
                    acc3 = wpool.tile([C, 512], F32, tag="acc3", bufs=2,
                                      name=f"acc3_{mh}")
                    nc.vector.tensor_add(out=acc3[:], in0=acc2[:],
                                         in1=z4[64 : 64 + C, :])
                    nc.vector.tensor_add(out=z_sb[:], in0=acc3[:],
                                         in1=z4[96 : 96 + C, :])
                else:
                    # general path: per-group E maps, sequential PSUM accum
                    for gi, (c0, c1, ag) in enumerate(groups):
                        gsz = c1 - c0
                        zps = ppool.tile([gsz, 512], F32, tag="zps", bufs=2,
                                         name=f"zps{mh}_{gi}")
                        for k in range(NT):
                            diff = wpool.tile([128, 512], F32, tag="diff",
                                              bufs=3, name=f"df{mh}_{gi}_{k}")
                            nc.gpsimd.tensor_scalar(
                                diff[:], xtb[mh][:], xc_pt[:, k : k + 1], None,
                                op0=mybir.AluOpType.subtract,
                            )
                            dsq = wpool.tile([128, 512], F32, tag="dsq",
                                             bufs=3, name=f"dq{mh}_{gi}_{k}")
                            nc.vector.tensor_mul(out=dsq[:], in0=diff[:],
                                                 in1=diff[:])
                            esb = wpool.tile([128, 512], F32, tag="esb",
                                             bufs=3, name=f"e{mh}_{gi}_{k}")
                            nc.scalar.activation(esb[:], dsq[:], Exp,
                                                 scale=-float(ag))
                            nc.tensor.matmul(
                                zps[:],
                                lhsT=r_t[k][:, c0:c1],
                                rhs=esb[:],
                                start=(k == 0),
                                stop=(k == NT - 1),
                            )
                        if c0 % 32 == 0:
                            nc.vector.tensor_copy(out=z_sb[c0:c1, :],
                                                  in_=zps[:])
                        else:
                            nc.sync.dma_start(out=z_sb[c0:c1, :], in_=zps[:])

                # ---- final linear; lin_b folded into the PSUM->SBUF copy ----
                for mt in range(MT):
                    ops = ppool.tile([128, OUT_C], F32, tag="smallps", bufs=2,
                                     name=f"ops{mh}_{mt}")
                    nc.tensor.matmul(
                        ops[:],
                        lhsT=z_sb[:, mt * 128 : (mt + 1) * 128],
                        rhs=wl[:],
                        start=True,
                        stop=True,
                    )
                    osb = wpool.tile([128, OUT_C], F32, tag="osb", bufs=3,
                                     name=f"o{mh}_{mt}")
                    nc.vector.tensor_add(out=osb[:], in0=ops[:], in1=blb[:])
                    m0 = mh * 512 + mt * 128
                    nc.sync.dma_start(out=y_out[m0 : m0 + 128, :], in_=osb[:])

    _split_multi_waits(nc)
    return nc


_cache = {}


def _get_nc(groups):
    key = tuple((c0, c1, np.float32(a).tobytes()) for c0, c1, a in groups)
    if key not in _cache:
        _cache[key] = _build(groups)
    return _cache[key]


def _prepare(r, x_context, y_context, x_target, conv_w, conv_b, sigma, lin_w,
             lin_b):
    r = np.asarray(r, np.float32)
    x_context = np.asarray(x_context, np.float32)
    x_target = np.asarray(x_target, np.float32)
    conv_w = np.asarray(conv_w, np.float32)
    conv_b = np.asarray(conv_b, np.float32)
    sigma = np.asarray(sigma, np.float32)
    lin_w = np.asarray(lin_w, np.float32)
    lin_b = np.asarray(lin_b, np.float32)

    # Channels sharing a length scale share one RBF map: sort channels by a,
    # group runs of equal values (uniform init sigma -> a single group).
    scales = np.exp(sigma.astype(np.float64))
    a = 0.5 / scales**2
    perm = np.argsort(a, kind="stable")
    a_s = a[perm]
    groups = []
    c0 = 0
    for c in range(1, C + 1):
        if c == C or a_s[c] != a_s[c0]:
            groups.append((c0, c, float(a_s[c0])))
            c0 = c
    groups = tuple(groups)

    # Repack weights (channel-permuted; conv bias row first, matching the
    # im2col ones row at partition 0).
    w_aug = np.concatenate(
        [conv_b[None, :], conv_w.transpose(2, 1, 0).reshape(C * KW, C)], axis=0
    )[:, perm]
    w_aug = np.ascontiguousarray(w_aug, np.float32)
    lin_w_t = np.ascontiguousarray(lin_w.T[perm], np.float32)
    lin_b_row = np.ascontiguousarray(lin_b[None, :], np.float32)

    in_maps = [
        {
            "r": np.ascontiguousarray(r[b]),
            "xc": np.ascontiguousarray(x_context[b].reshape(1, N_IN)),
            "xt": np.ascontiguousarray(x_target[b].reshape(1, N_OUT)),
            "w_aug": w_aug,
            "lin_w_t": lin_w_t,
            "lin_b": lin_b_row,
        }
        for b in range(B)
    ]
    return groups, in_maps


def kernel(**inputs):
    groups, in_maps = _prepare(**inputs)
    nc = _get_nc(groups)
    res = run_bass_kernel_spmd(nc, in_maps, list(range(N_CORES)))
    return np.stack([res.results[b]["y"] for b in range(B)], axis=0)


# revision 26
# speedup vs baseline: 2.2447x; 2.2447x over previous
"""ConvDecoder Bass kernel for Trainium2, SPMD over 8 NeuronCores.

Math (per batch element b, one per core):
    r_conv = Conv1d(r, conv_w, SAME) + conv_b            # (C, N_IN)
    d[n,m] = (xc[n] - xt[m])^2                           # (N_IN, N_OUT)
    wt_c   = exp(-0.5 * d / exp(sigma_c)^2)
    z[m,c] = sum_n r_conv[c,n] * wt_c[n,m]
    out    = z @ lin_w.T + lin_b                         # (N_OUT, OUT_C)

Per-core structure (v2):
  - Conv1d as an im2col matmul: ones row (bias) + 5 shifted DMA copies of r
    stacked on partitions -> (81, 512); matmul with repacked weights
    (81, 16) yields the conv output directly in (n, c) layout = the lhsT
    of the RBF-reduction matmul.
  - xt is partition-broadcast by a stride-0 DMA; GpSimd computes
    diff = xt - xc_p (per-partition scalar), DVE/ACT square it, ACT
    exponentiates with scale=-a (a = 0.5/scale^2, baked per group) ->
    E chunk (128, 512). fp32 end to end.
  - z[c,m] over the 4 n-tiles: 4 matmuls issued to 4 distinct PE column
    strips (tile_position) run concurrently in the array; DVE reduces the
    4 PSUM partials. (Channels sharing a length scale share one E map;
    with >1 sigma group, falls back to sequential PSUM accumulation.)
  - Final linear: (16,128)^T @ (16,32) matmul per m-tile; lin_b is folded
    into the PSUM->SBUF copy as a DVE add against a broadcast row.
"""

import numpy as np

import concourse.bass as bass
import concourse.mybir as mybir
from concourse.tile import TileContext
from concourse.bass_utils import run_bass_kernel_spmd

F32 = mybir.dt.float32

B, N_IN, N_OUT, C, OUT_C, KW = 8, 512, 1024, 16, 32, 5
N_CORES = 8
NT = N_IN // 128   # n tiles (4)
MH = N_OUT // 512  # m halves (2)
MT = 512 // 128    # m tiles per half (4)

# chunks (mh*NT+k) whose sub+square run fused on ACT (Square with
# per-partition bias) instead of DVE, to balance engine load against
# ACT's exp passes.
SQ_ON_ACT = {7}


# --- walrus workaround -----------------------------------------------------
# This container's walrus accepts at most ONE semaphore wait per TPB
# instruction, but Tile's scheduler attaches several (joins + tail drain).
# Hoist all but the last wait of each instruction onto fresh wait-only
# EventSemaphore instructions inserted right before it on the same engine.
_ws_ctr = [0]


def _split_multi_waits(nc):
    for fn in nc.m.functions:
        for blk in fn.blocks:
            insts = blk.instructions
            if not any(
                ins.sync_info and len(ins.sync_info.on_wait) > 1 for ins in insts
            ):
                continue
            out = []
            for ins in insts:
                si = ins.sync_info
                waits = list(si.on_wait) if si else []
                if len(waits) > 1:
                    for w in waits[:-1]:
                        _ws_ctr[0] += 1
                        ev = mybir.InstEventSemaphore(
                            name=f"waitsplit_{_ws_ctr[0]}", ins=[], outs=[]
                        )
                        ev.engine = ins.engine
                        ev.sync_info = mybir.SyncInfo(on_wait=[w], on_update=[])
                        nc.register_instruction(ev)
                        out.append(ev)
                    ins.sync_info = mybir.SyncInfo(
                        on_wait=[waits[-1]], on_update=list(si.on_update)
                    )
                out.append(ins)
            insts[:] = out


# --- kernel build ----------------------------------------------------------
def _build(groups):
    """groups: tuple of (c0, c1, a) with contiguous channel ranges."""
    nc = bass.Bass()
    r_in = nc.dram_tensor("r", [C, N_IN], F32, kind="ExternalInput")
    xc_in = nc.dram_tensor("xc", [1, N_IN], F32, kind="ExternalInput")
    xt_in = nc.dram_tensor("xt", [1, N_OUT], F32, kind="ExternalInput")
    wconv = nc.dram_tensor("w_aug", [C * KW + 1, C], F32, kind="ExternalInput")
    # lin128: lin_w_t at rows 32j+c, zeros elsewhere — the final matmul
    # contracts over the 4 z-strip partials and 16 channels in one go
    # (matmul cost is N-bound, so the 128-row contraction is free).
    wlin = nc.dram_tensor("lin128", [128, OUT_C], F32, kind="ExternalInput")
    blin = nc.dram_tensor("lin_b", [1, OUT_C], F32, kind="ExternalInput")
    y_out = nc.dram_tensor("y", [N_OUT, OUT_C], F32, kind="ExternalOutput")

    Exp = mybir.ActivationFunctionType.Exp
    Square = mybir.ActivationFunctionType.Square
    single_group = len(groups) == 1

    with TileContext(nc) as tc:
        with (
            tc.tile_pool(name="const", bufs=1) as cpool,
            tc.tile_pool(name="work", bufs=1) as wpool,
            tc.tile_pool(name="psum", bufs=1, space="PSUM") as ppool,
        ):
            # ---- constants / operand rows ----
            wa = cpool.tile([C * KW + 1, C], F32)
            nc.sync.dma_start(out=wa[:], in_=wconv[:])
            wl = cpool.tile([128, OUT_C], F32)
            nc.sync.dma_start(out=wl[:], in_=wlin[:])
            blb = cpool.tile([128, OUT_C], F32)
            nc.sync.dma_start(out=blb[:], in_=blin[0:1, :].partition_broadcast(128))

            # xc as per-partition scalars: xc_pt[p, t] = xc[t*128 + p]
            xc_pt = cpool.tile([128, NT], F32)
            nc.sync.dma_start(
                out=xc_pt[:], in_=xc_in[0, :].rearrange("(t p) -> p t", p=128)
            )
            neg_xc = cpool.tile([128, NT], F32)
            nc.vector.tensor_scalar_mul(neg_xc[:], xc_pt[:], -1.0)
            # xt broadcast to all partitions, one tile per m-half
            xtb = []
            for mh in range(MH):
                t = cpool.tile([128, 512], F32, name=f"xtb{mh}")
                nc.sync.dma_start(
                    out=t[:],
                    in_=xt_in[0:1, mh * 512 : (mh + 1) * 512].partition_broadcast(128),
                )
                xtb.append(t)

            # ---- conv im2col stack: row 0 = ones (bias), rows 1+16k+ci ----
            stack = cpool.tile([C * KW + 1, N_IN], F32)
            nc.vector.memset(stack[:, :], 0.0)
            pad = KW // 2
            for k in range(KW):
                lo = max(0, pad - k)
                hi = min(N_IN, N_IN + pad - k)
                nc.sync.dma_start(
                    out=stack[1 + C * k : 1 + C * (k + 1), lo:hi],
                    in_=r_in[:, lo + k - pad : hi + k - pad],
                )
            nc.vector.memset(stack[0:1, :], 1.0)

            # ---- conv matmuls: (81,128)^T @ (81,16) -> (128,16) per n-tile ----
            r_t = []
            for t in range(NT):
                cps = ppool.tile([128, C], F32, tag="smallps", bufs=2,
                                 name=f"cps{t}")
                nc.tensor.matmul(
                    cps[:],
                    lhsT=stack[:, t * 128 : (t + 1) * 128],
                    rhs=wa[:],
                    start=True,
                    stop=True,
                )
                rsb = cpool.tile([128, C], F32, name=f"rsb{t}")
                nc.vector.tensor_copy(out=rsb[:], in_=cps[:])
                r_t.append(rsb)

            # ---- main pipeline over m-halves / n-tiles ----
            for mh in range(MH):
                if single_group:
                    a0 = groups[0][2]
                    z4 = ppool.tile([128, 512], F32, tag="z4", bufs=2,
                                    name=f"z4_{mh}")
                    for k in range(NT):
                        dsq = wpool.tile([128, 512], F32, tag="dsq", bufs=3,
                                         name=f"dsq{mh}_{k}")
                        if mh * NT + k in SQ_ON_ACT:
                            # fused (xt - xc_p)^2 on ACT via per-partition bias
                            nc.scalar.activation(dsq[:], xtb[mh][:], Square,
                                                 bias=neg_xc[:, k : k + 1])
                        else:
                            diff = wpool.tile([128, 512], F32, tag="diff",
                                              bufs=3, name=f"diff{mh}_{k}")
                            nc.vector.tensor_scalar(
                                diff[:], xtb[mh][:], xc_pt[:, k : k + 1], None,
                                op0=mybir.AluOpType.subtract,
                            )
                            nc.vector.tensor_mul(out=dsq[:], in0=diff[:],
                                                 in1=diff[:])
                        esb = wpool.tile([128, 512], F32, tag="esb", bufs=3,
                                         name=f"e{mh}_{k}")
                        nc.scalar.activation(esb[:], dsq[:], Exp,
                                             scale=-float(a0))
                        # one PE column strip per n-tile: the 4 matmuls run
                        # concurrently in the array
                        nc.tensor.matmul(
                            z4[32 * k : 32 * k + C, :],
                            lhsT=r_t[k][:],
                            rhs=esb[:],
                            start=True,
                            stop=True,
                            tile_position=(0, 32 * k),
                        )
                    # copy the strips to SBUF at the same 32-aligned offsets
                    # (zeros elsewhere); the linear matmul below contracts
                    # over all 4 partials at no extra cost (N-bound)
                    zc = wpool.tile([128, 512], F32, tag="zc", bufs=2,
                                    name=f"zc{mh}")
                    nc.vector.memset(zc[:, :], 0.0)
                    for j in range(NT):
                        nc.vector.tensor_copy(
                            out=zc[32 * j : 32 * j + C, :],
                            in_=z4[32 * j : 32 * j + C, :],
                        )
                    for mt in range(MT):
                        ops = ppool.tile([128, OUT_C], F32, tag="smallps",
                                         bufs=2, name=f"ops{mh}_{mt}")
                        nc.tensor.matmul(
                            ops[:],
                            lhsT=zc[:, mt * 128 : (mt + 1) * 128],
                            rhs=wl[:],
                            start=True,
                            stop=True,
                        )
                        osb = wpool.tile([128, OUT_C], F32, tag="osb", bufs=3,
                                         name=f"o{mh}_{mt}")
                        nc.vector.tensor_add(out=osb[:], in0=ops[:],
                                             in1=blb[:])
                        m0 = mh * 512 + mt * 128
                        nc.sync.dma_start(out=y_out[m0 : m0 + 128, :],
                                          in_=osb[:])
                    continue

                z_sb = wpool.tile([C, 512], F32, tag="zsb", bufs=2,
                                  name=f"z{mh}")
                if True:
                    # general path: per-group E maps, sequential PSUM accum
                    for gi, (c0, c1, ag) in enumerate(groups):
                        gsz = c1 - c0
                        zps = ppool.tile([gsz, 512], F32, tag="zps", bufs=2,
                                         name=f"zps{mh}_{gi}")
                        for k in range(NT):
                            diff = wpool.tile([128, 512], F32, tag="diff",
                                              bufs=3, name=f"df{mh}_{gi}_{k}")
                            nc.vector.tensor_scalar(
                                diff[:], xtb[mh][:], xc_pt[:, k : k + 1], None,
                                op0=mybir.AluOpType.subtract,
                            )
                            dsq = wpool.tile([128, 512], F32, tag="dsq",
                                             bufs=3, name=f"dq{mh}_{gi}_{k}")
                            nc.vector.tensor_mul(out=dsq[:], in0=diff[:],
                                                 in1=diff[:])
                            esb = wpool.tile([128, 512], F32, tag="esb",
                                             bufs=3, name=f"e{mh}_{gi}_{k}")
                            nc.scalar.activation(esb[:], dsq[:], Exp,
                                                 scale=-float(ag))
                            nc.tensor.matmul(
                                zps[:],
                                lhsT=r_t[k][:, c0:c1],
                                rhs=esb[:],
                                start=(k == 0),
                                stop=(k == NT - 1),
                            )
                        if c0 % 32 == 0:
                            nc.vector.tensor_copy(out=z_sb[c0:c1, :],
                                                  in_=zps[:])
                        else:
                            nc.sync.dma_start(out=z_sb[c0:c1, :], in_=zps[:])

                # ---- final linear; lin_b folded into the PSUM->SBUF copy ----
                for mt in range(MT):
                    ops = ppool.tile([128, OUT_C], F32, tag="smallps", bufs=2,
                                     name=f"ops{mh}_{mt}")
                    nc.tensor.matmul(
                        ops[:],
                        lhsT=z_sb[:, mt * 128 : (mt + 1) * 128],
                        rhs=wl[0:C, :],
                        start=True,
                        stop=True,
                    )
                    osb = wpool.tile([128, OUT_C], F32, tag="osb", bufs=3,
                                     name=f"o{mh}_{mt}")
                    nc.vector.tensor_add(out=osb[:], in0=ops[:], in1=blb[:])
                    m0 = mh * 512 + mt * 128
                    nc.sync.dma_start(out=y_out[m0 : m0 + 128, :], in_=osb[:])

    _split_multi_waits(nc)
    return nc


_cache = {}


def _get_nc(groups):
    key = tuple((c0, c1, np.float32(a).tobytes()) for c0, c1, a in groups)
    if key not in _cache:
        _cache[key] = _build(groups)
    return _cache[key]


def _prepare(r, x_context, y_context, x_target, conv_w, conv_b, sigma, lin_w,
             lin_b):
    r = np.asarray(r, np.float32)
    x_context = np.asarray(x_context, np.float32)
    x_target = np.asarray(x_target, np.float32)
    conv_w = np.asarray(conv_w, np.float32)
    conv_b = np.asarray(conv_b, np.float32)
    sigma = np.asarray(sigma, np.float32)
    lin_w = np.asarray(lin_w, np.float32)
    lin_b = np.asarray(lin_b, np.float32)

    # Channels sharing a length scale share one RBF map: sort channels by a,
    # group runs of equal values (uniform init sigma -> a single group).
    scales = np.exp(sigma.astype(np.float64))
    a = 0.5 / scales**2
    perm = np.argsort(a, kind="stable")
    a_s = a[perm]
    groups = []
    c0 = 0
    for c in range(1, C + 1):
        if c == C or a_s[c] != a_s[c0]:
            groups.append((c0, c, float(a_s[c0])))
            c0 = c
    groups = tuple(groups)

    # Repack weights (channel-permuted; conv bias row first, matching the
    # im2col ones row at partition 0).
    w_aug = np.concatenate(
        [conv_b[None, :], conv_w.transpose(2, 1, 0).reshape(C * KW, C)], axis=0
    )[:, perm]
    w_aug = np.ascontiguousarray(w_aug, np.float32)
    lin_w_t = lin_w.T[perm]
    lin128 = np.zeros((128, OUT_C), np.float32)
    for j in range(4):
        lin128[32 * j : 32 * j + C] = lin_w_t
    lin_b_row = np.ascontiguousarray(lin_b[None, :], np.float32)

    in_maps = [
        {
            "r": np.ascontiguousarray(r[b]),
            "xc": np.ascontiguousarray(x_context[b].reshape(1, N_IN)),
            "xt": np.ascontiguousarray(x_target[b].reshape(1, N_OUT)),
            "w_aug": w_aug,
            "lin128": lin128,
            "lin_b": lin_b_row,
        }
        for b in range(B)
    ]
    return groups, in_maps


def kernel(**inputs):
    groups, in_maps = _prepare(**inputs)
    nc = _get_nc(groups)
    res = run_bass_kernel_spmd(nc, in_maps, list(range(N_CORES)))
    return np.stack([res.results[b]["y"] for b in range(B)], axis=0)


# revision 29
# speedup vs baseline: 2.3796x; 1.0601x over previous
"""ConvDecoder Bass kernel for Trainium2, SPMD over 8 NeuronCores.

Math (per batch element b, one per core):
    r_conv = Conv1d(r, conv_w, SAME) + conv_b            # (C, N_IN)
    d[n,m] = (xc[n] - xt[m])^2                           # (N_IN, N_OUT)
    wt_c   = exp(-0.5 * d / exp(sigma_c)^2)
    z[m,c] = sum_n r_conv[c,n] * wt_c[n,m]
    out    = z @ lin_w.T + lin_b                         # (N_OUT, OUT_C)

Per-core structure (v2):
  - Conv1d as an im2col matmul: ones row (bias) + 5 shifted DMA copies of r
    stacked on partitions -> (81, 512); matmul with repacked weights
    (81, 16) yields the conv output directly in (n, c) layout = the lhsT
    of the RBF-reduction matmul.
  - xt is partition-broadcast by a stride-0 DMA; GpSimd computes
    diff = xt - xc_p (per-partition scalar), DVE/ACT square it, ACT
    exponentiates with scale=-a (a = 0.5/scale^2, baked per group) ->
    E chunk (128, 512). fp32 end to end.
  - z[c,m] over the 4 n-tiles: 4 matmuls issued to 4 distinct PE column
    strips (tile_position) run concurrently in the array; DVE reduces the
    4 PSUM partials. (Channels sharing a length scale share one E map;
    with >1 sigma group, falls back to sequential PSUM accumulation.)
  - Final linear: (16,128)^T @ (16,32) matmul per m-tile; lin_b is folded
    into the PSUM->SBUF copy as a DVE add against a broadcast row.
"""

import numpy as np

import concourse.bass as bass
import concourse.mybir as mybir
from concourse.tile import TileContext
from concourse.bass_utils import run_bass_kernel_spmd

F32 = mybir.dt.float32

B, N_IN, N_OUT, C, OUT_C, KW = 8, 512, 1024, 16, 32, 5
N_CORES = 8
NT = N_IN // 128   # n tiles (4)
MH = N_OUT // 512  # m halves (2)
MT = 512 // 128    # m tiles per half (4)

# chunks (mh*NT+k) whose sub+square run fused on ACT (Square with
# per-partition bias) instead of DVE, to balance engine load against
# ACT's exp passes.
SQ_ON_ACT = {7}


# --- walrus workaround -----------------------------------------------------
# This container's walrus accepts at most ONE semaphore wait per TPB
# instruction, but Tile's scheduler attaches several (joins + tail drain).
# Hoist all but the last wait of each instruction onto fresh wait-only
# EventSemaphore instructions inserted right before it on the same engine.
_ws_ctr = [0]


def _split_multi_waits(nc):
    for fn in nc.m.functions:
        for blk in fn.blocks:
            insts = blk.instructions
            if not any(
                ins.sync_info and len(ins.sync_info.on_wait) > 1 for ins in insts
            ):
                continue
            out = []
            for ins in insts:
                si = ins.sync_info
                waits = list(si.on_wait) if si else []
                if len(waits) > 1:
                    for w in waits[:-1]:
                        _ws_ctr[0] += 1
                        ev = mybir.InstEventSemaphore(
                            name=f"waitsplit_{_ws_ctr[0]}", ins=[], outs=[]
                        )
                        ev.engine = ins.engine
                        ev.sync_info = mybir.SyncInfo(on_wait=[w], on_update=[])
                        nc.register_instruction(ev)
                        out.append(ev)
                    ins.sync_info = mybir.SyncInfo(
                        on_wait=[waits[-1]], on_update=list(si.on_update)
                    )
                out.append(ins)
            insts[:] = out


# --- kernel build ----------------------------------------------------------
def _build(groups):
    """groups: tuple of (c0, c1, a) with contiguous channel ranges."""
    nc = bass.Bass()
    r_in = nc.dram_tensor("r", [C, N_IN], F32, kind="ExternalInput")
    xc_in = nc.dram_tensor("xc", [1, N_IN], F32, kind="ExternalInput")
    xt_in = nc.dram_tensor("xt", [1, N_OUT], F32, kind="ExternalInput")
    wconv = nc.dram_tensor("w_aug", [C * KW + 1, C], F32, kind="ExternalInput")
    # lin128: lin_w_t at rows 32j+c, zeros elsewhere — the final matmul
    # contracts over the 4 z-strip partials and 16 channels in one go
    # (matmul cost is N-bound, so the 128-row contraction is free).
    wlin = nc.dram_tensor("lin128", [128, OUT_C], F32, kind="ExternalInput")
    blin = nc.dram_tensor("lin_b", [1, OUT_C], F32, kind="ExternalInput")
    y_out = nc.dram_tensor("y", [N_OUT, OUT_C], F32, kind="ExternalOutput")

    Exp = mybir.ActivationFunctionType.Exp
    Square = mybir.ActivationFunctionType.Square
    single_group = len(groups) == 1

    with TileContext(nc) as tc:
        with (
            tc.tile_pool(name="const", bufs=1) as cpool,
            tc.tile_pool(name="work", bufs=1) as wpool,
            tc.tile_pool(name="psum", bufs=1, space="PSUM") as ppool,
        ):
            # ---- inputs on the critical path first ----
            # xc as per-partition scalars: xc_pt[p, t] = xc[t*128 + p]
            xc_pt = cpool.tile([128, NT], F32)
            nc.sync.dma_start(
                out=xc_pt[:], in_=xc_in[0, :].rearrange("(t p) -> p t", p=128)
            )
            # xt broadcast to all partitions, one tile per m-half
            xtb = []
            for mh in range(MH):
                t = cpool.tile([128, 512], F32, name=f"xtb{mh}")
                nc.sync.dma_start(
                    out=t[:],
                    in_=xt_in[0:1, mh * 512 : (mh + 1) * 512].partition_broadcast(128),
                )
                xtb.append(t)
            # dummy exp: forces the ~1.3us ACT table load to run at t~=0
            # instead of stalling the first real exp mid-pipeline
            warm = cpool.tile([128, NT], F32)
            nc.scalar.activation(warm[:], xc_pt[:], Exp)
            neg_xc = cpool.tile([128, NT], F32)
            nc.vector.tensor_scalar_mul(neg_xc[:], xc_pt[:], -1.0)

            wa = cpool.tile([C * KW + 1, C], F32)
            nc.sync.dma_start(out=wa[:], in_=wconv[:])
            wl = cpool.tile([128, OUT_C], F32)
            nc.sync.dma_start(out=wl[:], in_=wlin[:])
            blb = cpool.tile([128, OUT_C], F32)
            nc.sync.dma_start(out=blb[:], in_=blin[0:1, :].partition_broadcast(128))

            # ---- conv im2col stack: row 0 = ones (bias), rows 1+16k+ci ----
            stack = cpool.tile([C * KW + 1, N_IN], F32)
            nc.vector.memset(stack[:, :], 0.0)
            pad = KW // 2
            for k in range(KW):
                lo = max(0, pad - k)
                hi = min(N_IN, N_IN + pad - k)
                nc.sync.dma_start(
                    out=stack[1 + C * k : 1 + C * (k + 1), lo:hi],
                    in_=r_in[:, lo + k - pad : hi + k - pad],
                )
            nc.vector.memset(stack[0:1, :], 1.0)

            # ---- conv matmuls: (81,128)^T @ (81,16) -> (128,16) per n-tile ----
            r_t = []
            for t in range(NT):
                cps = ppool.tile([128, C], F32, tag="smallps", bufs=2,
                                 name=f"cps{t}")
                nc.tensor.matmul(
                    cps[:],
                    lhsT=stack[:, t * 128 : (t + 1) * 128],
                    rhs=wa[:],
                    start=True,
                    stop=True,
                )
                # pad to 32 cols (zeros) so each z strip matmul writes a
                # full 32-partition group -> z4 has no undefined rows and
                # one whole-tile copy suffices
                rsb = cpool.tile([128, 2 * C], F32, name=f"rsb{t}")
                nc.vector.memset(rsb[:, C : 2 * C], 0.0)
                nc.vector.tensor_copy(out=rsb[:, 0:C], in_=cps[:])
                r_t.append(rsb)

            # ---- main pipeline over m-halves / n-tiles ----
            for mh in range(MH):
                if single_group:
                    a0 = groups[0][2]
                    z4 = ppool.tile([128, 512], F32, tag="z4", bufs=2,
                                    name=f"z4_{mh}")
                    for k in range(NT):
                        dsq = wpool.tile([128, 512], F32, tag="dsq", bufs=3,
                                         name=f"dsq{mh}_{k}")
                        if mh * NT + k in SQ_ON_ACT:
                            # fused (xt - xc_p)^2 on ACT via per-partition bias
                            nc.scalar.activation(dsq[:], xtb[mh][:], Square,
                                                 bias=neg_xc[:, k : k + 1])
                        else:
                            diff = wpool.tile([128, 512], F32, tag="diff",
                                              bufs=3, name=f"diff{mh}_{k}")
                            nc.vector.tensor_scalar(
                                diff[:], xtb[mh][:], xc_pt[:, k : k + 1], None,
                                op0=mybir.AluOpType.subtract,
                            )
                            nc.vector.tensor_mul(out=dsq[:], in0=diff[:],
                                                 in1=diff[:])
                        esb = wpool.tile([128, 512], F32, tag="esb", bufs=3,
                                         name=f"e{mh}_{k}")
                        nc.scalar.activation(esb[:], dsq[:], Exp,
                                             scale=-float(a0))
                        # one PE column strip per n-tile: the 4 matmuls run
                        # concurrently in the array
                        nc.tensor.matmul(
                            z4[32 * k : 32 * (k + 1), :],
                            lhsT=r_t[k][:],
                            rhs=esb[:],
                            start=True,
                            stop=True,
                            tile_position=(0, 32 * k),
                        )
                    # single whole-tile PSUM->SBUF copy; rows 32j+16..32j+31
                    # are computed zeros (padded lhsT), the linear matmul
                    # contracts over all 4 partials at no extra cost
                    zc = wpool.tile([128, 512], F32, tag="zc", bufs=2,
                                    name=f"zc{mh}")
                    nc.vector.tensor_copy(out=zc[:, :], in_=z4[:, :])
                    for mt in range(MT):
                        ops = ppool.tile([128, OUT_C], F32, tag="smallps",
                                         bufs=2, name=f"ops{mh}_{mt}")
                        nc.tensor.matmul(
                            ops[:],
                            lhsT=zc[:, mt * 128 : (mt + 1) * 128],
                            rhs=wl[:],
                            start=True,
                            stop=True,
                        )
                        osb = wpool.tile([128, OUT_C], F32, tag="osb", bufs=3,
                                         name=f"o{mh}_{mt}")
                        nc.vector.tensor_add(out=osb[:], in0=ops[:],
                                             in1=blb[:])
                        m0 = mh * 512 + mt * 128
                        nc.sync.dma_start(out=y_out[m0 : m0 + 128, :],
                                          in_=osb[:])
                    continue

                z_sb = wpool.tile([C, 512], F32, tag="zsb", bufs=2,
                                  name=f"z{mh}")
                if True:
                    # general path: per-group E maps, sequential PSUM accum
                    for gi, (c0, c1, ag) in enumerate(groups):
                        gsz = c1 - c0
                        zps = ppool.tile([gsz, 512], F32, tag="zps", bufs=2,
                                         name=f"zps{mh}_{gi}")
                        for k in range(NT):
                            diff = wpool.tile([128, 512], F32, tag="diff",
                                              bufs=3, name=f"df{mh}_{gi}_{k}")
                            nc.vector.tensor_scalar(
                                diff[:], xtb[mh][:], xc_pt[:, k : k + 1], None,
                                op0=mybir.AluOpType.subtract,
                            )
                            dsq = wpool.tile([128, 512], F32, tag="dsq",
                                             bufs=3, name=f"dq{mh}_{gi}_{k}")
                            nc.vector.tensor_mul(out=dsq[:], in0=diff[:],
                                                 in1=diff[:])
                            esb = wpool.tile([128, 512], F32, tag="esb",
                                             bufs=3, name=f"e{mh}_{gi}_{k}")
                            nc.scalar.activation(esb[:], dsq[:], Exp,
                                                 scale=-float(ag))
                            nc.tensor.matmul(
                                zps[:],
                                lhsT=r_t[k][:, c0:c1],
                                rhs=esb[:],
                                start=(k == 0),
                                stop=(k == NT - 1),
                            )
                        if c0 % 32 == 0:
                            nc.vector.tensor_copy(out=z_sb[c0:c1, :],
                                                  in_=zps[:])
                        else:
                            nc.sync.dma_start(out=z_sb[c0:c1, :], in_=zps[:])

                # ---- final linear; lin_b folded into the PSUM->SBUF copy ----
                for mt in range(MT):
                    ops = ppool.tile([128, OUT_C], F32, tag="smallps", bufs=2,
                                     name=f"ops{mh}_{mt}")
                    nc.tensor.matmul(
                        ops[:],
                        lhsT=z_sb[:, mt * 128 : (mt + 1) * 128],
                        rhs=wl[0:C, :],
                        start=True,
                        stop=True,
                    )
                    osb = wpool.tile([128, OUT_C], F32, tag="osb", bufs=3,
                                     name=f"o{mh}_{mt}")
                    nc.vector.tensor_add(out=osb[:], in0=ops[:], in1=blb[:])
                    m0 = mh * 512 + mt * 128
                    nc.sync.dma_start(out=y_out[m0 : m0 + 128, :], in_=osb[:])

    _split_multi_waits(nc)
    return nc


_cache = {}


def _get_nc(groups):
    key = tuple((c0, c1, np.float32(a).tobytes()) for c0, c1, a in groups)
    if key not in _cache:
        _cache[key] = _build(groups)
    return _cache[key]


def _prepare(r, x_context, y_context, x_target, conv_w, conv_b, sigma, lin_w,
             lin_b):
    r = np.asarray(r, np.float32)
    x_context = np.asarray(x_context, np.float32)
    x_target = np.asarray(x_target, np.float32)
    conv_w = np.asarray(conv_w, np.float32)
    conv_b = np.asarray(conv_b, np.float32)
    sigma = np.asarray(sigma, np.float32)
    lin_w = np.asarray(lin_w, np.float32)
    lin_b = np.asarray(lin_b, np.float32)

    # Channels sharing a length scale share one RBF map: sort channels by a,
    # group runs of equal values (uniform init sigma -> a single group).
    scales = np.exp(sigma.astype(np.float64))
    a = 0.5 / scales**2
    perm = np.argsort(a, kind="stable")
    a_s = a[perm]
    groups = []
    c0 = 0
    for c in range(1, C + 1):
        if c == C or a_s[c] != a_s[c0]:
            groups.append((c0, c, float(a_s[c0])))
            c0 = c
    groups = tuple(groups)

    # Repack weights (channel-permuted; conv bias row first, matching the
    # im2col ones row at partition 0).
    w_aug = np.concatenate(
        [conv_b[None, :], conv_w.transpose(2, 1, 0).reshape(C * KW, C)], axis=0
    )[:, perm]
    w_aug = np.ascontiguousarray(w_aug, np.float32)
    lin_w_t = lin_w.T[perm]
    lin128 = np.zeros((128, OUT_C), np.float32)
    for j in range(4):
        lin128[32 * j : 32 * j + C] = lin_w_t
    lin_b_row = np.ascontiguousarray(lin_b[None, :], np.float32)

    in_maps = [
        {
            "r": np.ascontiguousarray(r[b]),
            "xc": np.ascontiguousarray(x_context[b].reshape(1, N_IN)),
            "xt": np.ascontiguousarray(x_target[b].reshape(1, N_OUT)),
            "w_aug": w_aug,
            "lin128": lin128,
            "lin_b": lin_b_row,
        }
        for b in range(B)
    ]
    return groups, in_maps


def kernel(**inputs):
    groups, in_maps = _prepare(**inputs)
    nc = _get_nc(groups)
    res = run_bass_kernel_spmd(nc, in_maps, list(range(N_CORES)))
    return np.stack([res.results[b]["y"] for b in range(B)], axis=0)
